# revision 11
# baseline (speedup 1.0000x reference)
"""Trainium2 Bass kernel for nn_BatchedChebLayer (gnn_message_passing) — v2.

Strategy (8 NeuronCores, SPMD, bf16 data path / f32 psum):
- Flatten features: h = x.transpose(1,0,2).reshape(N, 512), cast bf16.
- Chebyshev: out = x@W0' + S1@W1 + S2@W2', W0'=W0-W2, W2'=2*W2 (host fold),
  S1 = L@h, S2 = L@S1.
- Row sharding: core c owns 49 windows x 128 rows (degree-balanced snake
  dealing). Each SPMM hop: per window, batched dma_gather (up to 1024 rows
  per call, bf16 1KB rows) + per-128-token-block sel matmul into a psum
  [128,512] accumulator. int16 gather indices -> table split at row 32768
  (lo/hi tables, per-block half assignment).
- hop1 program: gathers from h, writes S1 windows (bf16, window-local order).
- hop2 program: gathers from host-reassembled S1 table, then fuses the whole
  Chebyshev dense stage per window: TensorE-transpose of S2 slices + 12
  [128x128x128] matmuls with host-pretransposed x/S1 window tiles -> out
  rows directly (f32). No separate dense launch.
"""
import sys
sys.path.insert(0, "/opt/trn_rl_repo")
sys.path.insert(0, "/root/.axon_site/_ro/trn_rl_repo")
import numpy as np
import ml_dtypes

BF16 = ml_dtypes.bfloat16
T, N, E, C, KCH = 4, 50000, 800000, 128, 3
D = T * C                    # 512
NCORES = 8
NWIN = 49                    # windows (psum tiles) per core
RPAD = NWIN * 128            # 6272 padded rows per core
SPLIT = 32768                # int16 index limit -> lo/hi table split
NHI = N - SPLIT              # 17232
MAXG = 8                     # blocks (x128 tokens) per dma_gather call

_cache = {}


def make_calls(bw):
    """bw: [NWIN, 2] block counts. Returns per-window call lists
    [(half, G, idx_slot_base)] plus NB (total blocks) and total idx slots."""
    calls = []
    slot = 0
    nb = 0
    for w in range(NWIN):
        wcalls = []
        for half in (0, 1):
            b = int(bw[w, half])
            while b > 0:
                G = min(MAXG, b)
                wcalls.append((half, G, slot))
                slot += G * 8
                nb += G
                b -= G
        calls.append(wcalls)
    return calls, nb, slot


def build_schedule(edge_row, edge_col, edge_val):
    deg = np.bincount(edge_row, minlength=N)
    deg_lo = np.bincount(edge_row[edge_col < SPLIT], minlength=N)
    deg_hi = deg - deg_lo
    srows = np.argsort(-deg, kind="stable")
    nbuck = NCORES * NWIN
    pos = np.arange(N)
    p_, j_ = pos // nbuck, pos % nbuck
    buck = np.where(p_ % 2 == 0, j_, nbuck - 1 - j_)
    win_of = np.empty(N, np.int64)
    win_of[srows] = buck // NCORES

    # greedy core assignment within each window: balance (lo, hi) token
    # counts across cores, capacity 128 rows per (core, window)
    core_of = np.empty(N, np.int64)
    m_of = np.empty(N, np.int64)
    order_w = np.argsort(win_of * N - deg, kind="stable")  # by win, deg desc
    wstarts = np.searchsorted(win_of[order_w], np.arange(NWIN + 1))
    for w in range(NWIN):
        rws = order_w[wstarts[w]:wstarts[w + 1]]
        lo_l = np.zeros(NCORES)
        hi_l = np.zeros(NCORES)
        nrow = np.zeros(NCORES, np.int64)
        for r in rws:
            score = np.maximum(lo_l + deg_lo[r], np.max(lo_l)) + \
                np.maximum(hi_l + deg_hi[r], np.max(hi_l))
            score = np.where(nrow >= 128, np.inf, score)
            c = int(np.argmin(score))
            core_of[r] = c
            m_of[r] = nrow[c]
            lo_l[c] += deg_lo[r]
            hi_l[c] += deg_hi[r]
            nrow[c] += 1
    row_of = np.full((NCORES, RPAD), -1, np.int64)
    row_of[core_of, win_of * 128 + m_of] = np.arange(N)

    rows = edge_row.astype(np.int64)
    cols = edge_col.astype(np.int64)
    vals = edge_val.astype(np.float32)
    tcore_e = core_of[rows]
    twin_e = win_of[rows]
    tm_e = m_of[rows]
    thalf_e = (cols >= SPLIT).astype(np.int64)

    # dedup: tokens with identical (core, win, half, col) share one gather
    # slot; their sel column carries multiple nonzeros
    gid_e = ((tcore_e * NWIN + twin_e) * 2 + thalf_e)
    key = gid_e * N + cols
    ukey, tok_of_e = np.unique(key, return_inverse=True)
    ucols = ukey % N
    gid = ukey // N
    tcore = gid // (NWIN * 2)
    twin = (gid // 2) % NWIN
    thalf = gid % 2
    tidx = ucols - SPLIT * thalf
    NTOK = len(ukey)

    ngroups = NCORES * NWIN * 2
    # ukey sorted -> tokens already grouped by gid
    gstarts = np.searchsorted(gid, np.arange(ngroups), side="left")
    q = np.arange(NTOK) - gstarts[gid]
    cnt = np.bincount(gid, minlength=ngroups).reshape(NCORES, NWIN, 2)
    bw = np.ceil(cnt.max(axis=0) / 128).astype(np.int64)   # [NWIN, 2]
    bw = np.maximum(bw, (cnt.max(axis=0) > 0))
    # ensure at least one block per window for psum init
    none = bw.sum(axis=1) == 0
    bw[none, 0] = 1

    calls, NB, nslots = make_calls(bw)

    blk_lo = bw[:, 0]
    tb = bw.sum(axis=1)
    win_blk_base = np.zeros(NWIN + 1, np.int64)
    np.cumsum(tb, out=win_blk_base[1:])

    block_local = q // 128
    tp = q % 128
    bl_window = block_local + np.where(thalf == 1, blk_lo[twin], 0)
    B = win_blk_base[twin] + bl_window

    # idx slot: per (win, half): chunk k = block_local//8, g = block_local%8
    # slot = call_base[win,half,k] + g*8 + tp//16 ; partition tp%16
    maxch = int(np.ceil(bw.max() / MAXG)) if bw.max() > 0 else 1
    call_base = np.zeros((NWIN, 2, maxch), np.int64)
    for w in range(NWIN):
        for (half, G, base) in calls[w]:
            # chunk index = how many calls of this half precede it
            pass
    # fill call_base properly
    chcount = np.zeros((NWIN, 2), np.int64)
    for w in range(NWIN):
        for (half, G, base) in calls[w]:
            call_base[w, half, chcount[w, half]] = base
            chcount[w, half] += 1
    k = block_local // MAXG
    g = block_local % MAXG
    slot = call_base[twin, thalf, k] + g * 8 + tp // 16

    # scatter per-edge values into the (deduped) token slots
    sel_f32 = np.zeros((NCORES, 128, NB * 128), np.float32)
    np.add.at(sel_f32, (tcore_e, tp[tok_of_e],
                        B[tok_of_e] * 128 + tm_e), vals)
    sel_all = sel_f32.astype(BF16)
    idx16 = np.zeros((NCORES, 16, nslots), np.int16)
    idx16[tcore, tp % 16, slot] = tidx.astype(np.int16)
    idx_all = np.tile(idx16, (1, 8, 1))

    return bw, calls, NB, nslots, row_of, sel_all, idx_all


def _build_hop(bw, calls, NB, nslots, hop2, reps=1):
    import concourse.bacc as bacc
    import concourse.tile as tile
    import concourse.mybir as mybir

    bf = mybir.dt.bfloat16
    f32 = mybir.dt.float32
    nc = bacc.Bacc("TRN2", target_bir_lowering=False, debug=False,
                   num_devices=NCORES)
    tlo = nc.dram_tensor("tlo", [SPLIT, D], bf, kind="ExternalInput")
    thi = nc.dram_tensor("thi", [NHI, D], bf, kind="ExternalInput")
    idx = nc.dram_tensor("idx", [128, nslots], mybir.dt.int16,
                         kind="ExternalInput")
    sel = nc.dram_tensor("sel", [128, NB * 128], bf, kind="ExternalInput")
    if hop2:
        xT4 = nc.dram_tensor("xT4", [128, NWIN * D], bf, kind="ExternalInput")
        s1T4 = nc.dram_tensor("s1T4", [128, NWIN * D], bf, kind="ExternalInput")
        wf = nc.dram_tensor("wf", [128, T * KCH * C], bf, kind="ExternalInput")
        ident = nc.dram_tensor("ident", [128, 128], bf, kind="ExternalInput")
        outw = nc.dram_tensor("outw", [RPAD, D], f32, kind="ExternalOutput")
    else:
        s1o = nc.dram_tensor("s1o", [RPAD, D], bf, kind="ExternalOutput")

    tabs = [tlo, thi]
    win_blk_base = np.zeros(NWIN + 1, np.int64)
    np.cumsum(bw.sum(axis=1), out=win_blk_base[1:])

    with tile.TileContext(nc) as tc:
        with (
            tc.tile_pool(name="const", bufs=1) as cpool,
            tc.tile_pool(name="gp", bufs=10) as gpool,
            tc.tile_pool(name="selp", bufs=4) as selpool,
            tc.tile_pool(name="stg", bufs=3) as spool,
            tc.tile_pool(name="ps", bufs=2, space="PSUM") as ppool,
            tc.tile_pool(name="ps2", bufs=2, space="PSUM") as p2pool,
        ):
            idx_t = cpool.tile([128, nslots], mybir.dt.int16)
            nc.sync.dma_start(idx_t[:], idx[:])
            if hop2:
                wf_t = cpool.tile([128, T * KCH * C], bf)
                nc.sync.dma_start(wf_t[:], wf[:])
                id_t = cpool.tile([128, 128], bf)
                nc.sync.dma_start(id_t[:], ident[:])
            with tc.For_i(0, reps) as _r:
                for w in range(NWIN):
                    tb = int(bw[w].sum())
                    b0 = int(win_blk_base[w])
                    sel_t = selpool.tile([128, tb * 128], bf, tag="sel")
                    nc.sync.dma_start(sel_t[:],
                                      sel[:, b0 * 128:(b0 + tb) * 128])
                    gts = []
                    for (half, G, base) in calls[w]:
                        gt = gpool.tile([128, MAXG, D], bf, tag="g")
                        nc.gpsimd.dma_gather(
                            gt[:, :G, :], tabs[half][:],
                            idx_t[:, base:base + G * 8],
                            G * 128, G * 128, D)
                        gts.append((gt, G))
                    ps = ppool.tile([128, D], f32)
                    bl = 0
                    for (gt, G) in gts:
                        for gg in range(G):
                            nc.tensor.matmul(
                                out=ps[:],
                                lhsT=sel_t[:, bl * 128:(bl + 1) * 128],
                                rhs=gt[:, gg, :],
                                start=(bl == 0), stop=(bl == tb - 1))
                            bl += 1
                    if not hop2:
                        st = spool.tile([128, D], bf, tag="st")
                        nc.vector.tensor_copy(st[:], ps[:])
                        nc.sync.dma_start(s1o[w * 128:(w + 1) * 128, :], st[:])
                    else:
                        s2sb = spool.tile([128, D], bf, tag="s2sb")
                        nc.vector.tensor_copy(s2sb[:], ps[:])
                        xs = spool.tile([128, D], bf, tag="xs")
                        nc.sync.dma_start(xs[:], xT4[:, w * D:(w + 1) * D])
                        s1s = spool.tile([128, D], bf, tag="s1s")
                        nc.sync.dma_start(s1s[:], s1T4[:, w * D:(w + 1) * D])
                        s2t = spool.tile([128, D], bf, tag="s2t")
                        for t in range(T):
                            pst = p2pool.tile([128, 128], bf, tag="pst")
                            nc.tensor.transpose(
                                pst[:], s2sb[:, t * 128:(t + 1) * 128], id_t[:])
                            nc.vector.tensor_copy(
                                s2t[:, t * 128:(t + 1) * 128], pst[:])
                        po = p2pool.tile([128, D], f32, tag="po")
                        for t in range(T):
                            sl = slice(t * 128, (t + 1) * 128)
                            for k2, src in enumerate((xs, s1s, s2t)):
                                nc.tensor.matmul(
                                    out=po[:, sl],
                                    lhsT=src[:, sl],
                                    rhs=wf_t[:, (t * KCH + k2) * 128:
                                             (t * KCH + k2 + 1) * 128],
                                    start=(k2 == 0), stop=(k2 == KCH - 1))
                        so = spool.tile([128, D], f32, tag="so")
                        nc.vector.tensor_copy(so[:], po[:])
                        nc.sync.dma_start(outw[w * 128:(w + 1) * 128, :], so[:])
    nc.compile()
    return nc


def _pack_T4(a):
    """[RPAD, 512] -> [128, NWIN*512] with [p, w*512 + t*128 + r] = a[w*128+r,
    t*128+p] (per-window feature-transposed tiles, t-major)."""
    a4 = np.ascontiguousarray(a.reshape(NWIN, 128, T, 128).transpose(3, 0, 2, 1))
    return a4.reshape(128, NWIN * D)


def kernel(x, edge_row, edge_col, edge_val, weight, bias):
    from concourse.bass_utils import run_bass_kernel_spmd

    x = np.asarray(x, dtype=np.float32)
    edge_row = np.asarray(edge_row).astype(np.int64)
    edge_col = np.asarray(edge_col).astype(np.int64)
    edge_val = np.asarray(edge_val, dtype=np.float32)
    weight = np.asarray(weight, dtype=np.float32)
    bias = np.asarray(bias, dtype=np.float32)

    fp = (edge_row[::997].tobytes(), edge_col[::997].tobytes())
    if _cache.get("fp") != fp:
        for k_ in ("sched", "hop1", "hop2"):
            _cache.pop(k_, None)
        _cache["fp"] = fp
    if "sched" not in _cache:
        _cache["sched"] = build_schedule(edge_row, edge_col, edge_val)
    bw, calls, NB, nslots, row_of, sel_all, idx_all = _cache["sched"]
    if "hop1" not in _cache:
        _cache["hop1"] = _build_hop(bw, calls, NB, nslots, hop2=False)
    if "hop2" not in _cache:
        _cache["hop2"] = _build_hop(bw, calls, NB, nslots, hop2=True)

    h = np.ascontiguousarray(
        x.transpose(1, 0, 2).reshape(N, D)).astype(BF16)      # [N, 512]
    valid = row_of >= 0
    clamped = np.maximum(row_of, 0)

    ins1 = [{"tlo": h[:SPLIT], "thi": h[SPLIT:],
             "idx": idx_all[c], "sel": sel_all[c]} for c in range(NCORES)]
    r1 = run_bass_kernel_spmd(_cache["hop1"], ins1, core_ids=list(range(NCORES)))
    s1_win = [r1.results[c]["s1o"] for c in range(NCORES)]    # [RPAD, 512] bf16

    s1_glob = np.zeros((N, D), BF16)
    for c in range(NCORES):
        s1_glob[row_of[c][valid[c]]] = s1_win[c][valid[c]]

    wfold = np.stack([weight[:, 0] - weight[:, 2], weight[:, 1],
                      2.0 * weight[:, 2]], axis=1)            # [T,3,C,C] in,out
    wf_arr = np.ascontiguousarray(
        wfold.transpose(2, 0, 1, 3).reshape(C, T * KCH * C)).astype(BF16)
    ident = np.eye(128, dtype=np.float32).astype(BF16)

    ins2 = []
    for c in range(NCORES):
        hw_ = np.where(valid[c][:, None], h[clamped[c]].astype(np.float32), 0.0)
        ins2.append({
            "tlo": s1_glob[:SPLIT], "thi": s1_glob[SPLIT:],
            "idx": idx_all[c], "sel": sel_all[c],
            "xT4": _pack_T4(hw_.astype(BF16)),
            "s1T4": _pack_T4(np.asarray(s1_win[c])),
            "wf": wf_arr, "ident": ident,
        })
    r2 = run_bass_kernel_spmd(_cache["hop2"], ins2, core_ids=list(range(NCORES)))

    out = np.empty((T, N, C), np.float32)
    for c in range(NCORES):
        ow = r2.results[c]["outw"]                            # [RPAD, 512] f32
        for t in range(T):
            out[t, row_of[c][valid[c]], :] = ow[valid[c], t * 128:(t + 1) * 128]
    out += bias[:, None, :]
    return out
